# revision 2
# baseline (speedup 1.0000x reference)
"""AtomSelectionModel (GINE message passing + scatter softmax) on 8 trn2 cores.

Strategy: node-sharded (32768 nodes/core, edge -> core of dst). Device kernels:
  K_PRE : node embedding x0 = relu([x_upd, Z[g]] @ W_emb + b) and edge
          embedding e = relu(edge_attr @ W_edge + b), both as feat-major
          matmul stacks on PE.
  K_MSG : per layer - dma_gather x[src] rows, add e, relu, dma_scatter_add
          into agg by dst (unique-dst chunk packing, per-chunk valid-count
          registers, trailing -1 pads).
  K_UPD : per layer - h = relu(W1.T(x+agg)+b1); x += relu(W2.T h + b2).
  K_HEAD: logits via MLP head.
  K_SMAX: per-graph masked softmax on a [graph, slot] grid.
Host does only sharding, layout transposes, and index plumbing between
launches; every FLOP of the model runs on device.
"""
import numpy as np

V = 262144
E = 524288
NG = 8192
FV = 64
FE = 16
H = 128
NL = 4
W = 8
VC = V // W          # 32768 nodes per core
NCHG = 10            # chunks per src-group
CHUNK = 1024
NCH = W * NCHG       # 80 chunks per core per layer
EPAD = NCH * CHUNK   # 81920 edge slots per core
GPC = NG // W        # 1024 graphs per core

# ----------------------------------------------------------------------------
# walrus sync-wait cap workaround: spill >1 sem waits onto injected nops
# ----------------------------------------------------------------------------
_tilefix_done = [False]


def _install_tilefix():
    if _tilefix_done[0]:
        return
    _tilefix_done[0] = True
    import bass_rust
    import concourse.mybir as mybir
    import concourse.tile as tile

    WAIT_CAP = 1
    nid = [0]

    def _spill(nc):
        for f in nc.m.functions:
            for bb in f.blocks:
                live = bb.instructions
                out = []
                changed = False
                for ins in live:
                    si = ins.sync_info
                    waits = list(si.on_wait) if si and si.on_wait else []
                    if len(waits) > WAIT_CAP:
                        changed = True
                        keep = waits[:WAIT_CAP]
                        rest = waits[WAIT_CAP:]
                        for i in range(0, len(rest), WAIT_CAP):
                            nid[0] += 1
                            nop = bass_rust.InstNoOp(
                                name=f"WSPILL-{nid[0]}", ins=[], outs=[])
                            nop.engine = ins.engine
                            nop.sync_info = mybir.SyncInfo(
                                on_wait=rest[i:i + WAIT_CAP], on_update=[])
                            out.append(nop)
                            nc.register_instruction(nop, overwrite=True)
                        si.on_wait = keep
                    out.append(ins)
                if changed:
                    live[:] = out

    orig_exit = tile.TileContext.__exit__

    def _exit(self, *a, **k):
        r = orig_exit(self, *a, **k)
        _spill(self.nc)
        return r

    tile.TileContext.__exit__ = _exit


# ----------------------------------------------------------------------------
# reusable PJRT runner (jit built once per kernel, reused across calls)
# ----------------------------------------------------------------------------
class Runner:
    def __init__(self, nc, n_cores=W):
        import jax
        import concourse.mybir as mybir
        from concourse import bass2jax
        from jax.sharding import Mesh, PartitionSpec
        from jax.experimental.shard_map import shard_map

        bass2jax.install_neuronx_cc_hook()
        self.nc = nc
        self.n = n_cores
        in_names, out_names, out_avals, zero_outs = [], [], [], []
        pname = nc.partition_id_tensor.name if nc.partition_id_tensor else None
        for alloc in nc.m.functions[0].allocations:
            if not isinstance(alloc, mybir.MemoryLocationSet):
                continue
            name = alloc.memorylocations[0].name
            if alloc.kind == "ExternalInput":
                if name != pname:
                    in_names.append(name)
            elif alloc.kind == "ExternalOutput":
                shape = tuple(alloc.tensor_shape)
                dt = mybir.dt.np(alloc.dtype)
                out_names.append(name)
                out_avals.append(jax.core.ShapedArray(shape, dt))
                zero_outs.append(np.zeros(shape, dt))
        self.in_names, self.out_names = in_names, out_names
        self.out_avals, self.zero_outs = out_avals, zero_outs
        n_params = len(in_names)
        n_outs = len(out_avals)
        all_names = list(in_names) + list(out_names)
        if pname is not None:
            all_names.append(pname)
        donate = tuple(range(n_params, n_params + n_outs))

        def _body(*args):
            operands = list(args)
            if pname is not None:
                operands.append(bass2jax.partition_id_tensor())
            outs = bass2jax._bass_exec_p.bind(
                *operands,
                out_avals=tuple(out_avals),
                in_names=tuple(all_names),
                out_names=tuple(out_names),
                lowering_input_output_aliases=(),
                sim_require_finite=True,
                sim_require_nnan=True,
                nc=nc,
            )
            return tuple(outs)

        devices = jax.devices()[:n_cores]
        mesh = Mesh(np.asarray(devices), ("core",))
        in_specs = (PartitionSpec("core"),) * (n_params + n_outs)
        out_specs = (PartitionSpec("core"),) * n_outs
        self.fn = jax.jit(
            shard_map(_body, mesh=mesh, in_specs=in_specs,
                      out_specs=out_specs, check_rep=False),
            donate_argnums=donate, keep_unused=True)

    def __call__(self, in_maps):
        per_core = [[np.asarray(m[k]) for k in self.in_names] for m in in_maps]
        concat_in = [np.concatenate([per_core[c][i] for c in range(self.n)], 0)
                     for i in range(len(self.in_names))]
        concat_zeros = [np.zeros((self.n * z.shape[0],) + z.shape[1:], z.dtype)
                        for z in self.zero_outs]
        outs = self.fn(*concat_in, *concat_zeros)
        res = []
        for c in range(self.n):
            d = {}
            for i, name in enumerate(self.out_names):
                a = np.asarray(outs[i])
                d[name] = a.reshape((self.n,) + self.out_avals[i].shape)[c]
            res.append(d)
        return res


# ----------------------------------------------------------------------------
# device kernel builders
# ----------------------------------------------------------------------------
def _bass_mods():
    _install_tilefix()
    import concourse.bass as bass
    import concourse.mybir as mybir
    import concourse.tile as tile
    return bass, mybir, tile


def build_pre():
    bass, mybir, tile = _bass_mods()
    f32 = mybir.dt.float32
    nc = bass.Bass()
    catT = nc.dram_tensor("catT", [H + 256, VC], f32, kind="ExternalInput")
    wemb = nc.dram_tensor("wemb", [H + 256, H], f32, kind="ExternalInput")
    bemb = nc.dram_tensor("bemb", [H, 1], f32, kind="ExternalInput")
    eaT = nc.dram_tensor("eaT", [FE, EPAD], f32, kind="ExternalInput")
    wedge = nc.dram_tensor("wedge", [FE, H], f32, kind="ExternalInput")
    bedge = nc.dram_tensor("bedge", [H, 1], f32, kind="ExternalInput")
    x0T = nc.dram_tensor("x0T", [H, VC], f32, kind="ExternalOutput")
    eT = nc.dram_tensor("eT", [H, EPAD], f32, kind="ExternalOutput")
    NT = 512
    with tile.TileContext(nc) as tc:
        with tc.tile_pool(name="w", bufs=1) as wp, \
             tc.tile_pool(name="io", bufs=3) as io, \
             tc.tile_pool(name="ps", bufs=2, space="PSUM") as ps:
            wt = []
            for k in range(3):
                t = wp.tile([128, H], f32, tag=f"w{k}")
                nc.sync.dma_start(out=t[:], in_=wemb.ap()[k * 128:(k + 1) * 128, :])
                wt.append(t)
            bt = wp.tile([H, 1], f32, tag="bemb")
            nc.sync.dma_start(out=bt[:], in_=bemb.ap())
            we = wp.tile([FE, H], f32, tag="wedge")
            nc.sync.dma_start(out=we[:], in_=wedge.ap())
            be = wp.tile([H, 1], f32, tag="bedge")
            nc.sync.dma_start(out=be[:], in_=bedge.ap())
            for n0 in range(0, VC, NT):
                acc = ps.tile([128, NT], f32, tag="acc")
                for k in range(3):
                    rt = io.tile([128, NT], f32, tag="rt")
                    nc.sync.dma_start(out=rt[:], in_=catT.ap()[k * 128:(k + 1) * 128, n0:n0 + NT])
                    nc.tensor.matmul(acc[:], wt[k][:], rt[:], start=(k == 0), stop=(k == 2))
                ot = io.tile([128, NT], f32, tag="ot")
                nc.scalar.activation(out=ot[:], in_=acc[:],
                                     func=mybir.ActivationFunctionType.Relu,
                                     bias=bt[:])
                nc.sync.dma_start(out=x0T.ap()[:, n0:n0 + NT], in_=ot[:])
            for n0 in range(0, EPAD, NT):
                acc = ps.tile([128, NT], f32, tag="acc2")
                rt = io.tile([FE, NT], f32, tag="rte")
                nc.sync.dma_start(out=rt[:], in_=eaT.ap()[:, n0:n0 + NT])
                nc.tensor.matmul(acc[:], we[:], rt[:], start=True, stop=True)
                ot = io.tile([128, NT], f32, tag="ote")
                nc.scalar.activation(out=ot[:], in_=acc[:],
                                     func=mybir.ActivationFunctionType.Relu,
                                     bias=be[:])
                nc.sync.dma_start(out=eT.ap()[:, n0:n0 + NT], in_=ot[:])
    return nc


def build_msg():
    bass, mybir, tile = _bass_mods()
    from concourse import library_config
    f32 = mybir.dt.float32
    i16 = mybir.dt.int16
    nc = bass.Bass()
    xf = nc.dram_tensor("xf", [V, H], f32, kind="ExternalInput")
    etok = nc.dram_tensor("etok", [128, NCH * (CHUNK // 128) * H], f32, kind="ExternalInput")
    gidx = nc.dram_tensor("gidx", [128, NCH * CHUNK // 16], i16, kind="ExternalInput")
    didx = nc.dram_tensor("didx", [128, NCH * CHUNK // 16], i16, kind="ExternalInput")
    cnts = nc.dram_tensor("cnts", [1, NCH], mybir.dt.int32, kind="ExternalInput")
    agg = nc.dram_tensor("agg", [VC, H], f32, kind="ExternalOutput")
    with tile.TileContext(nc) as tc:
        nc.gpsimd.load_library(library_config.mlp)
        reg = nc.gpsimd.alloc_register("nval")
        with tc.tile_pool(name="p", bufs=4) as pool, \
             tc.tile_pool(name="pi", bufs=1) as ipool:
            gi = ipool.tile([128, NCH * CHUNK // 16], i16, tag="gi")
            di = ipool.tile([128, NCH * CHUNK // 16], i16, tag="di")
            cn = ipool.tile([1, NCH], mybir.dt.int32, tag="cn")
            nc.sync.dma_start(out=gi[:], in_=gidx.ap())
            nc.sync.dma_start(out=di[:], in_=didx.ap())
            nc.sync.dma_start(out=cn[:], in_=cnts.ap())
            for p in range(W):
                src_tab = xf.ap()[p * VC:(p + 1) * VC, :]
                for q in range(NCHG):
                    ch = p * NCHG + q
                    cs = slice(ch * (CHUNK // 16), (ch + 1) * (CHUNK // 16))
                    nc.gpsimd.reg_load(reg, cn[0:1, ch:ch + 1])
                    g = pool.tile([128, CHUNK // 128, H], f32, tag="g")
                    nc.gpsimd.dma_gather(g[:], src_tab, gi[:, cs], CHUNK, reg, H)
                    e = pool.tile([128, CHUNK // 128, H], f32, tag="e")
                    nc.sync.dma_start(
                        out=e[:],
                        in_=etok.ap()[:, ch * (CHUNK // 128) * H:(ch + 1) * (CHUNK // 128) * H])
                    nc.vector.tensor_add(out=g[:], in0=g[:], in1=e[:])
                    nc.scalar.activation(out=g[:], in_=g[:],
                                         func=mybir.ActivationFunctionType.Relu)
                    nc.gpsimd.dma_scatter_add(agg.ap(), g[:], di[:, cs], CHUNK, reg, H)
    from concourse.library_overlay import lower_extended_insts
    lower_extended_insts(nc)
    return nc


def build_upd():
    bass, mybir, tile = _bass_mods()
    f32 = mybir.dt.float32
    nc = bass.Bass()
    xT = nc.dram_tensor("xT", [H, VC], f32, kind="ExternalInput")
    aT = nc.dram_tensor("aT", [H, VC], f32, kind="ExternalInput")
    w1 = nc.dram_tensor("w1", [H, H], f32, kind="ExternalInput")
    b1 = nc.dram_tensor("b1", [H, 1], f32, kind="ExternalInput")
    w2 = nc.dram_tensor("w2", [H, H], f32, kind="ExternalInput")
    b2 = nc.dram_tensor("b2", [H, 1], f32, kind="ExternalInput")
    xo = nc.dram_tensor("xo", [H, VC], f32, kind="ExternalOutput")
    NT = 512
    with tile.TileContext(nc) as tc:
        with tc.tile_pool(name="w", bufs=1) as wp, \
             tc.tile_pool(name="io", bufs=3) as io, \
             tc.tile_pool(name="ps", bufs=2, space="PSUM") as ps:
            w1t = wp.tile([H, H], f32, tag="w1")
            w2t = wp.tile([H, H], f32, tag="w2")
            b1t = wp.tile([H, 1], f32, tag="b1")
            b2t = wp.tile([H, 1], f32, tag="b2")
            nc.sync.dma_start(out=w1t[:], in_=w1.ap())
            nc.sync.dma_start(out=w2t[:], in_=w2.ap())
            nc.sync.dma_start(out=b1t[:], in_=b1.ap())
            nc.sync.dma_start(out=b2t[:], in_=b2.ap())
            for n0 in range(0, VC, NT):
                tx = io.tile([128, NT], f32, tag="tx")
                ta = io.tile([128, NT], f32, tag="ta")
                nc.sync.dma_start(out=tx[:], in_=xT.ap()[:, n0:n0 + NT])
                nc.sync.dma_start(out=ta[:], in_=aT.ap()[:, n0:n0 + NT])
                nc.vector.tensor_add(out=ta[:], in0=ta[:], in1=tx[:])
                hp = ps.tile([128, NT], f32, tag="hp")
                nc.tensor.matmul(hp[:], w1t[:], ta[:], start=True, stop=True)
                hs = io.tile([128, NT], f32, tag="hs")
                nc.scalar.activation(out=hs[:], in_=hp[:],
                                     func=mybir.ActivationFunctionType.Relu,
                                     bias=b1t[:])
                up = ps.tile([128, NT], f32, tag="up")
                nc.tensor.matmul(up[:], w2t[:], hs[:], start=True, stop=True)
                us = io.tile([128, NT], f32, tag="us")
                nc.scalar.activation(out=us[:], in_=up[:],
                                     func=mybir.ActivationFunctionType.Relu,
                                     bias=b2t[:])
                nc.vector.tensor_add(out=us[:], in0=us[:], in1=tx[:])
                nc.sync.dma_start(out=xo.ap()[:, n0:n0 + NT], in_=us[:])
    return nc


def build_head():
    bass, mybir, tile = _bass_mods()
    f32 = mybir.dt.float32
    nc = bass.Bass()
    xcT = nc.dram_tensor("xcT", [H + FV, VC], f32, kind="ExternalInput")
    wm1 = nc.dram_tensor("wm1", [H + FV, H], f32, kind="ExternalInput")
    bm1 = nc.dram_tensor("bm1", [H, 1], f32, kind="ExternalInput")
    wm2 = nc.dram_tensor("wm2", [H, 1], f32, kind="ExternalInput")
    bm2 = nc.dram_tensor("bm2", [1, 1], f32, kind="ExternalInput")
    lg = nc.dram_tensor("lg", [1, VC], f32, kind="ExternalOutput")
    NT = 512
    with tile.TileContext(nc) as tc:
        with tc.tile_pool(name="w", bufs=1) as wp, \
             tc.tile_pool(name="io", bufs=3) as io, \
             tc.tile_pool(name="ps", bufs=2, space="PSUM") as ps:
            wa = wp.tile([128, H], f32, tag="wa")
            wb = wp.tile([FV, H], f32, tag="wb")
            nc.sync.dma_start(out=wa[:], in_=wm1.ap()[0:128, :])
            nc.sync.dma_start(out=wb[:], in_=wm1.ap()[128:128 + FV, :])
            b1t = wp.tile([H, 1], f32, tag="bm1")
            nc.sync.dma_start(out=b1t[:], in_=bm1.ap())
            w2t = wp.tile([H, 1], f32, tag="wm2")
            nc.sync.dma_start(out=w2t[:], in_=wm2.ap())
            b2t = wp.tile([1, 1], f32, tag="bm2")
            nc.sync.dma_start(out=b2t[:], in_=bm2.ap())
            for n0 in range(0, VC, NT):
                ra = io.tile([128, NT], f32, tag="ra")
                rb = io.tile([FV, NT], f32, tag="rb")
                nc.sync.dma_start(out=ra[:], in_=xcT.ap()[0:128, n0:n0 + NT])
                nc.sync.dma_start(out=rb[:], in_=xcT.ap()[128:128 + FV, n0:n0 + NT])
                hp = ps.tile([128, NT], f32, tag="hp")
                nc.tensor.matmul(hp[:], wa[:], ra[:], start=True, stop=False)
                nc.tensor.matmul(hp[:], wb[:], rb[:], start=False, stop=True)
                hs = io.tile([128, NT], f32, tag="hs")
                nc.scalar.activation(out=hs[:], in_=hp[:],
                                     func=mybir.ActivationFunctionType.Relu,
                                     bias=b1t[:])
                lp = ps.tile([1, NT], f32, tag="lp")
                nc.tensor.matmul(lp[:], w2t[:], hs[:], start=True, stop=True)
                ls = io.tile([1, NT], f32, tag="ls")
                nc.vector.tensor_scalar_add(ls[:], lp[:], b2t[:])
                nc.sync.dma_start(out=lg.ap()[:, n0:n0 + NT], in_=ls[:])
    return nc


def build_smax(gmax):
    bass, mybir, tile = _bass_mods()
    f32 = mybir.dt.float32
    nc = bass.Bass()
    NGRP = GPC // 128  # 8
    grid = nc.dram_tensor("grid", [128, NGRP * gmax], f32, kind="ExternalInput")
    prob = nc.dram_tensor("prob", [128, NGRP * gmax], f32, kind="ExternalOutput")
    with tile.TileContext(nc) as tc:
        with tc.tile_pool(name="p", bufs=2) as pool:
            for j in range(NGRP):
                t = pool.tile([128, gmax], f32, tag="t")
                nc.sync.dma_start(out=t[:], in_=grid.ap()[:, j * gmax:(j + 1) * gmax])
                m = pool.tile([128, 1], f32, tag="m")
                nc.vector.tensor_reduce(m[:], t[:], mybir.AxisListType.X,
                                        mybir.AluOpType.max)
                nc.vector.tensor_scalar_sub(t[:], t[:], m[:])
                nc.scalar.activation(out=t[:], in_=t[:],
                                     func=mybir.ActivationFunctionType.Exp)
                s = pool.tile([128, 1], f32, tag="s")
                nc.vector.tensor_reduce(s[:], t[:], mybir.AxisListType.X,
                                        mybir.AluOpType.add)
                r = pool.tile([128, 1], f32, tag="r")
                nc.vector.reciprocal(r[:], s[:])
                nc.vector.tensor_scalar_mul(t[:], t[:], r[:])
                nc.sync.dma_start(out=prob.ap()[:, j * gmax:(j + 1) * gmax], in_=t[:])
    return nc


# ----------------------------------------------------------------------------
# host-side prep
# ----------------------------------------------------------------------------
def _wrap16(a):
    """[n] int16 idx list -> [128, n/16] wrapped layout (16 rows replicated x8)."""
    w = a.reshape(-1, 16).T
    return np.tile(w, (8, 1)).astype(np.int16)


def _pack_edges(src, dst):
    """Per core: chunk assignment with unique dst per chunk.
    Returns per-core dicts with gidx, didx, cnts, eperm (slot -> edge id)."""
    cores = []
    co = dst // VC
    grp = src // VC
    for c in range(W):
        gi = np.full((NCH, CHUNK), -1, np.int16)
        di = np.full((NCH, CHUNK), -1, np.int16)
        cn = np.zeros(NCH, np.int32)
        eperm = np.full((NCH, CHUNK), -1, np.int64)
        for p in range(W):
            sel = np.nonzero((co == c) & (grp == p))[0]
            d = dst[sel] - c * VC
            # occurrence index per dst (sorted by dst)
            order = np.argsort(d, kind="stable")
            ds = d[order]
            occ = np.arange(len(ds)) - np.searchsorted(ds, ds, side="left")
            chunk = (ds.astype(np.int64) + occ) % NCHG
            assert occ.max(initial=0) < NCHG, "degree exceeds NCHG"
            for q in range(NCHG):
                m = chunk == q
                k = int(m.sum())
                assert k <= CHUNK, f"chunk overflow {k}"
                ch = p * NCHG + q
                eids = sel[order[m]]
                gi[ch, :k] = (src[eids] - p * VC).astype(np.int16)
                di[ch, :k] = (dst[eids] - c * VC).astype(np.int16)
                eperm[ch, :k] = eids
                cn[ch] = k
        cores.append(dict(gidx=_wrap16(gi.ravel()), didx=_wrap16(di.ravel()),
                          cnts=cn[None], eperm=eperm.ravel()))
    return cores


def _tok_layout(a):
    """[EPAD, H] -> token-major [128, NCH*(CHUNK//128)*H]."""
    t = a.reshape(NCH, CHUNK // 128, 128, H).transpose(2, 0, 1, 3)
    return np.ascontiguousarray(t.reshape(128, -1))


_runners = {}


def _get_runner(name, builder):
    if name not in _runners:
        _runners[name] = Runner(builder())
    return _runners[name]


def kernel(x_inp_core, edge_index_core, edge_attr_core, x_upd_core, Z_core,
           Z_block, node2graph_core, W_emb, b_emb, W_edge, b_edge,
           W1_layers, b1_layers, W2_layers, b2_layers,
           W_mlp1, b_mlp1, W_mlp2, b_mlp2):
    import time
    t_dev = 0.0
    x_inp = np.asarray(x_inp_core, np.float32)
    ei = np.asarray(edge_index_core, np.int64)
    ea = np.asarray(edge_attr_core, np.float32)
    x_upd = np.asarray(x_upd_core, np.float32)
    Zc = np.asarray(Z_core, np.float32)
    Zb = np.asarray(Z_block, np.float32)
    n2g = np.asarray(node2graph_core, np.int64)
    src, dst = ei[0], ei[1]

    packs = _pack_edges(src, dst)
    Zcat = np.concatenate([Zc, Zb], 1)          # (NG, 256)
    Zg = Zcat[n2g]                               # (V, 256) host indexing

    # ---- K_PRE ----
    r_pre = _get_runner("pre", build_pre)
    ins = []
    for c in range(W):
        sl = slice(c * VC, (c + 1) * VC)
        catT = np.ascontiguousarray(
            np.concatenate([x_upd[sl], Zg[sl]], 1).T)   # (384, VC)
        ea_slot = np.zeros((EPAD, FE), np.float32)
        ep = packs[c]["eperm"]
        m = ep >= 0
        ea_slot[m] = ea[ep[m]]
        ins.append(dict(catT=catT, wemb=W_emb.astype(np.float32),
                        bemb=b_emb.reshape(H, 1).astype(np.float32),
                        eaT=np.ascontiguousarray(ea_slot.T),
                        wedge=W_edge.astype(np.float32),
                        bedge=b_edge.reshape(H, 1).astype(np.float32)))
    t0 = time.time()
    outs = r_pre(ins)
    t_dev += time.time() - t0
    xT = [outs[c]["x0T"] for c in range(W)]               # feat-major per core
    etok = [_tok_layout(np.ascontiguousarray(outs[c]["eT"].T)) for c in range(W)]

    # ---- layers ----
    r_msg = _get_runner("msg", build_msg)
    r_upd = _get_runner("upd", build_upd)
    W1 = np.asarray(W1_layers, np.float32)
    B1 = np.asarray(b1_layers, np.float32)
    W2 = np.asarray(W2_layers, np.float32)
    B2 = np.asarray(b2_layers, np.float32)
    for l in range(NL):
        xfull = np.concatenate([np.ascontiguousarray(xT[c].T) for c in range(W)], 0)
        ins = [dict(xf=xfull, etok=etok[c], gidx=packs[c]["gidx"],
                    didx=packs[c]["didx"], cnts=packs[c]["cnts"])
               for c in range(W)]
        t0 = time.time()
        outs = r_msg(ins)
        t_dev += time.time() - t0
        ins = [dict(xT=xT[c], aT=np.ascontiguousarray(outs[c]["agg"].T),
                    w1=W1[l], b1=B1[l].reshape(H, 1),
                    w2=W2[l], b2=B2[l].reshape(H, 1)) for c in range(W)]
        t0 = time.time()
        outs = r_upd(ins)
        t_dev += time.time() - t0
        xT = [outs[c]["xo"] for c in range(W)]

    # ---- head ----
    r_head = _get_runner("head", build_head)
    ins = []
    for c in range(W):
        sl = slice(c * VC, (c + 1) * VC)
        xcT = np.concatenate([xT[c], np.ascontiguousarray(x_inp[sl].T)], 0)
        ins.append(dict(xcT=xcT, wm1=W_mlp1.astype(np.float32),
                        bm1=b_mlp1.reshape(H, 1).astype(np.float32),
                        wm2=W_mlp2.astype(np.float32),
                        bm2=np.asarray(b_mlp2, np.float32).reshape(1, 1)))
    t0 = time.time()
    outs = r_head(ins)
    t_dev += time.time() - t0
    logit = np.concatenate([outs[c]["lg"][0] for c in range(W)])  # (V,)

    # ---- scatter softmax on [graph, slot] grid ----
    counts = np.bincount(n2g, minlength=NG)
    gmax = int(counts.max())
    gmax = max(32, int(np.ceil(gmax / 32) * 32))
    NGRP = GPC // 128
    # node order within each graph (n2g sorted)
    starts = np.zeros(NG + 1, np.int64)
    np.cumsum(counts, out=starts[1:])
    slot_in_g = np.arange(V) - starts[n2g]
    r_smax = _get_runner(f"smax{gmax}", lambda: build_smax(gmax))
    ins = []
    for c in range(W):
        grid = np.full((128, NGRP, gmax), -1e30, np.float32)
        gsel = (n2g >= c * GPC) & (n2g < (c + 1) * GPC)
        gl = n2g[gsel] - c * GPC
        grid[gl % 128, gl // 128, slot_in_g[gsel]] = logit[gsel]
        ins.append(dict(grid=grid.reshape(128, NGRP * gmax)))
    t0 = time.time()
    outs = r_smax(ins)
    t_dev += time.time() - t0
    P = np.zeros(V, np.float32)
    for c in range(W):
        pg = outs[c]["prob"].reshape(128, NGRP, gmax)
        gsel = (n2g >= c * GPC) & (n2g < (c + 1) * GPC)
        gl = n2g[gsel] - c * GPC
        P[gsel] = pg[gl % 128, gl // 128, slot_in_g[gsel]]
    kernel._t_dev = t_dev
    return P
